# revision 8
# baseline (speedup 1.0000x reference)
"""Trainium2 Bass kernel for nn_AttentionAggregator.

Math (per node n, K=32 neighbors, D=128):
  q = self @ W1; m = neigh @ W2; t = sigmoid(dt*W3)
  score = relu((q+m+t) @ W4 + b4); alpha = softmax_k(score)
  out = sum_k alpha * neigh;  returns (out, alpha)

Key factoring: (q+m+t)@W4 = self@(W1@W4) + neigh@(W2@W4) + f(dt)
  where f(x) = sum_e sigmoid(x*W3[e])*W4[e]  (scalar function, poly-fit on host).
So the only full-size work on the big [N,K,D] tensor is:
  s2 = neigh . v2  (DVE mult + segmented reduce), and
  out = alpha-weighted sum over k (PE block-diag matmuls with PSUM accum).
Data-parallel over nodes across 8 cores; everything per-core is independent.
"""

import hashlib
import os
import shutil
import numpy as np

import concourse.bass as bass
import concourse.tile as tile
from concourse import bacc, mybir
from concourse.bass_utils import run_bass_kernel_spmd

FP32 = mybir.dt.float32
AX = mybir.AxisListType
OP = mybir.AluOpType
AF = mybir.ActivationFunctionType

N_FULL, K, D = 50000, 32, 128
NCORES = 8
NPC = 6272            # nodes per core (pad 50000 -> 50176)
DEG = 14              # poly degree for f(dt)

# ---------------------------------------------------------------- NEFF cache
_CACHE_DIR = os.path.join(os.path.expanduser("~"), ".bass_neff_cache")


def _install_neff_cache():
    import concourse.bass2jax as b2j
    if getattr(b2j, "_neff_cache_installed", False):
        return
    os.makedirs(_CACHE_DIR, exist_ok=True)
    real = b2j.compile_bir_kernel

    def cached(ant_bir_str, compile_dir_path, neff_name="kernel.neff", **kw):
        key = hashlib.sha256(
            ant_bir_str if isinstance(ant_bir_str, bytes) else ant_bir_str.encode()
        ).hexdigest()[:32]
        cpath = os.path.join(_CACHE_DIR, f"{key}.neff")
        dst = os.path.join(compile_dir_path, neff_name)
        if os.path.exists(cpath):
            shutil.copy(cpath, dst)
            return dst
        f = real(ant_bir_str, compile_dir_path, neff_name=neff_name, **kw)
        try:
            shutil.copy(f, cpath)
        except OSError:
            pass
        return f

    b2j.compile_bir_kernel = cached
    b2j._neff_cache_installed = True


# ---------------------------------------------------------------- program
def build_program(npc=NPC, ct=56):
    """One SPMD program, identical on all cores. npc nodes/core, ct nk-tiles
    per chunk. Requires npc % 32 == 0, ct % 8 == 0."""
    assert npc % 32 == 0 and ct % 8 == 0
    nkpc = npc * K
    nt = nkpc // 128          # nk-tiles of 128 rows
    assert nt % 8 == 0
    ngrp = nt // 8            # groups of 8 tiles = 32 nodes
    u_full, u_rem = npc // 128, npc % 128   # node-tiles for self
    dtf, dt_remp = nkpc // 16384, (nkpc % 16384) // 128

    nc = bacc.Bacc("TRN2", target_bir_lowering=False, debug=False,
                   num_devices=NCORES)

    neigh = nc.dram_tensor("neigh", [nkpc, D], FP32, kind="ExternalInput").ap()
    selfv = nc.dram_tensor("selfv", [npc, D], FP32, kind="ExternalInput").ap()
    dt_in = nc.dram_tensor("dt", [nkpc], FP32, kind="ExternalInput").ap()
    v1rep = nc.dram_tensor("v1rep", [128, D], FP32, kind="ExternalInput").ap()
    v2rep = nc.dram_tensor("v2rep", [128, D], FP32, kind="ExternalInput").ap()
    coef = nc.dram_tensor("coef", [128, DEG + 1], FP32, kind="ExternalInput").ap()
    e4 = nc.dram_tensor("e4", [128, 4], FP32, kind="ExternalInput").ap()
    e4b = nc.dram_tensor("e4b", [4, 128], FP32, kind="ExternalInput").ap()
    eye = nc.dram_tensor("eye", [128, 128], FP32, kind="ExternalInput").ap()

    cscratch = nc.dram_tensor("cscratch", [npc], FP32).ap()
    out = nc.dram_tensor("out", [npc, D], FP32, kind="ExternalOutput").ap()
    score = nc.dram_tensor("score", [nkpc], FP32, kind="ExternalOutput").ap()

    with tile.TileContext(nc) as tc:
        with tc.tile_pool(name="persist", bufs=1) as pp:
            # ---- constants to SBUF
            v1s = pp.tile([128, D], FP32)
            nc.sync.dma_start(v1s[:], v1rep[:])
            v2s = pp.tile([128, D], FP32)
            nc.sync.dma_start(v2s[:], v2rep[:])
            cfs = pp.tile([128, DEG + 1], FP32)
            nc.sync.dma_start(cfs[:], coef[:])
            e4s = pp.tile([128, 4], FP32)
            nc.sync.dma_start(e4s[:], e4[:])
            e4bs = pp.tile([4, 128], FP32)
            nc.sync.dma_start(e4bs[:], e4b[:])
            eyes = pp.tile([128, 128], FP32)
            nc.sync.dma_start(eyes[:], eye[:])

            gsb = pp.tile([128, nt], FP32)       # g = c_rep + f(dt) + a0, score layout
            a_bufs = [pp.tile([128, (ct // 8) * 256], FP32, tag=f"A{i}",
                              name=f"Abuf{i}") for i in range(2)]

            # ================= phase 0: g = poly(dt^T) + broadcast(self.v1) =====
            with tc.tile_pool(name="ph0", bufs=1) as p0, \
                 tc.tile_pool(name="ph0ps", bufs=1, space="PSUM") as p0ps:
                # ---- c = self . v1  (per node)
                u_cols = u_full + (1 if u_rem else 0)
                csb = p0.tile([128, u_cols], FP32, tag="cnat")
                nc.vector.memset(csb[:], 0.0)
                if u_full:
                    ssb = p0.tile([128, u_full * D], FP32, tag="selfsb")
                    nc.sync.dma_start(
                        ssb[:].rearrange("p (u d) -> p u d", d=D),
                        selfv[: u_full * 128].rearrange("(u p) d -> p u d", p=128))
                    smul = p0.tile([128, u_full * D], FP32, tag="smul")
                    nc.vector.tensor_tensor(
                        smul[:].rearrange("p (u d) -> p u d", d=D),
                        ssb[:].rearrange("p (u d) -> p u d", d=D),
                        v1s[:].unsqueeze(1).broadcast_to([128, u_full, D]),
                        op=OP.mult)
                    nc.vector.tensor_reduce(
                        csb[:, :u_full],
                        smul[:].rearrange("p (u d) -> p u d", d=D),
                        axis=AX.X, op=OP.add)
                if u_rem:
                    ssb2 = p0.tile([u_rem, D], FP32, tag="selfsb2")
                    nc.sync.dma_start(ssb2[:], selfv[u_full * 128:])
                    smul2 = p0.tile([u_rem, D], FP32, tag="smul2")
                    nc.vector.tensor_tensor(smul2[:], ssb2[:], v1s[:u_rem, :],
                                            op=OP.mult)
                    nc.vector.tensor_reduce(
                        csb[:u_rem, u_full:u_full + 1], smul2[:],
                        axis=AX.X, op=OP.add)
                # c4[j, t] = c[4t + j] via DRAM scratch (simple DMAs)
                if u_full:
                    nc.sync.dma_start(
                        cscratch[: u_full * 128].rearrange("(u p) -> p u", p=128),
                        csb[:, :u_full])
                if u_rem:
                    nc.sync.dma_start(cscratch[u_full * 128:],
                                      csb[:u_rem, u_full:u_full + 1])
                c4 = p0.tile([4, (nt + 31) // 32 * 32], FP32, tag="c4")
                nc.sync.dma_start(
                    c4[:, :nt],
                    cscratch[:].rearrange("(t j) -> j t", j=4))

                # ---- dt transpose to score layout, 128x128 blocks via PE
                pdt = p0ps.tile([128, 128], FP32, tag="pdt")
                dtx = p0.tile([128, nt], FP32, tag="dtx")
                if dtf:
                    dtn = p0.tile([128, dtf * 128], FP32, tag="dtn")
                    nc.sync.dma_start(
                        dtn[:].rearrange("p (t c) -> p t c", c=128),
                        dt_in[: dtf * 16384].rearrange("(t p c) -> p t c",
                                                       p=128, c=128))
                    for t in range(dtf):
                        pdt_t = p0ps.tile([128, 128], FP32, tag="pdt")
                        nc.tensor.matmul(pdt_t[:], dtn[:, t * 128:(t + 1) * 128],
                                         eyes[:], start=True, stop=True)
                        nc.scalar.copy(dtx[:, t * 128:(t + 1) * 128], pdt_t[:])
                if dt_remp:
                    dtn2 = p0.tile([dt_remp, 128], FP32, tag="dtn2")
                    nc.sync.dma_start(
                        dtn2[:],
                        dt_in[dtf * 16384:].rearrange("(p c) -> p c", c=128))
                    pdt2 = p0ps.tile([128, dt_remp], FP32, tag="pdt2")
                    nc.tensor.matmul(pdt2[:], dtn2[:],
                                     eyes[:dt_remp, :dt_remp],
                                     start=True, stop=True)
                    nc.scalar.copy(dtx[:, dtf * 128: dtf * 128 + dt_remp],
                                   pdt2[:])

                # ---- c broadcast to [128, nt] via PE: lhsT=e4b, rhs=c4
                pg = p0ps.tile([128, ((nt + 511) // 512) * 512], FP32, tag="pg")
                for s0 in range(0, nt, 512):
                    s1 = min(s0 + 512, nt)
                    nc.tensor.matmul(pg[:, s0:s1], e4bs[:], c4[:, s0:s1],
                                     start=True, stop=True)

                # ---- poly eval: r=0; r=(r+b_j)*x ...; g = (r + a0) + c_rep
                r0 = p0.tile([128, nt], FP32, tag="poly0")
                r1 = p0.tile([128, nt], FP32, tag="poly1")
                nc.vector.tensor_scalar_mul(r0[:], dtx[:], cfs[:, 0:1])
                cur, nxt = r0, r1
                for j in range(1, DEG):
                    nc.vector.scalar_tensor_tensor(
                        nxt[:], cur[:], cfs[:, j:j + 1], dtx[:],
                        op0=OP.add, op1=OP.mult)
                    cur, nxt = nxt, cur
                nc.vector.scalar_tensor_tensor(
                    gsb[:], cur[:], cfs[:, DEG:DEG + 1], pg[:, :nt],
                    op0=OP.add, op1=OP.add)

            # ================= phase 1: main streaming loop ====================
            with tc.tile_pool(name="nb", bufs=3) as nbp, \
                 tc.tile_pool(name="nv", bufs=2) as nvp, \
                 tc.tile_pool(name="sm", bufs=3) as smp, \
                 tc.tile_pool(name="osb", bufs=2) as osbp, \
                 tc.tile_pool(name="ps", bufs=2, space="PSUM") as psp, \
                 tc.tile_pool(name="pso", bufs=2, space="PSUM") as psop:

                for ab in a_bufs:
                    nc.vector.memset(ab[:], 0.0)

                nchunks = (nt + ct - 1) // ct
                gglob = 0           # global group index
                fill_nodes = 0      # nodes accumulated in current out batch
                osb = None
                osb_base = 0        # first node row of current osb batch
                OSB_FILLS = 8
                out_ps = None

                for ck in range(nchunks):
                    t0 = ck * ct
                    ctc = min(ct, nt - t0)          # tiles this chunk
                    ng_c = ctc // 8                 # groups this chunk
                    nb = nbp.tile([128, ct * D], FP32, tag="nb")
                    nc.sync.dma_start(
                        nb[:, :ctc * D].rearrange("p (t d) -> p t d", d=D),
                        neigh[t0 * 128:(t0 + ctc) * 128].rearrange(
                            "(t p) d -> p t d", p=128))

                    # ---- s2 = neigh . v2 (two halves)
                    s2 = smp.tile([128, ct], FP32, tag="s2")
                    half = ctc // 2
                    for h, (ha, hb) in enumerate(((0, half), (half, ctc))):
                        nh = hb - ha
                        nv = nvp.tile([128, (ct // 2 + 4) * D], FP32, tag="nv")
                        nc.vector.tensor_tensor(
                            nv[:, :nh * D].rearrange("p (t d) -> p t d", d=D),
                            nb[:, ha * D:hb * D].rearrange("p (t d) -> p t d", d=D),
                            v2s[:].unsqueeze(1).broadcast_to([128, nh, D]),
                            op=OP.mult)
                        nc.vector.tensor_reduce(
                            s2[:, ha:hb],
                            nv[:, :nh * D].rearrange("p (t d) -> p t d", d=D),
                            axis=AX.X, op=OP.add)

                    # ---- scores: spre = relu(s2 + g); e = exp(spre)
                    spre = smp.tile([128, ct], FP32, tag="spre")
                    nc.vector.tensor_tensor(spre[:, :ctc], s2[:, :ctc],
                                            gsb[:, t0:t0 + ctc], op=OP.add)
                    sprer = smp.tile([128, ct], FP32, tag="sprer")
                    nc.vector.tensor_scalar_max(sprer[:, :ctc], spre[:, :ctc], 0.0)
                    esb = smp.tile([128, ct], FP32, tag="esb")
                    nc.scalar.activation(esb[:, :ctc], sprer[:, :ctc], AF.Exp)

                    # ---- softmax denom: sums over k via PE mask matmul
                    ps_s = psp.tile([4, ct], FP32, tag="ps_s")
                    nc.tensor.matmul(ps_s[:, :ctc], e4s[:], esb[:, :ctc],
                                     start=True, stop=True)
                    inv = smp.tile([4, ct], FP32, tag="inv")
                    nc.vector.reciprocal(inv[:, :ctc], ps_s[:, :ctc])
                    ps_b = psp.tile([128, ct], FP32, tag="ps_b")
                    nc.tensor.matmul(ps_b[:, :ctc], e4bs[:], inv[:, :ctc],
                                     start=True, stop=True)
                    alpha = smp.tile([128, ct], FP32, tag="alpha")
                    nc.vector.tensor_tensor(alpha[:, :ctc], esb[:, :ctc],
                                            ps_b[:, :ctc], op=OP.mult)

                    # ---- score out
                    nc.sync.dma_start(
                        score[t0 * 128:(t0 + ctc) * 128].rearrange(
                            "(t p) -> p t", p=128),
                        alpha[:, :ctc])

                    # ---- A build: A[r, 256g + 36t + q] = alpha[r, 8g+t], q=r//32
                    A = a_bufs[ck % 2]
                    for q in range(4):
                        nc.vector.tensor_copy(
                            A[32 * q:32 * (q + 1), :ng_c * 256].rearrange(
                                "p (g j) -> p g j", j=256)[:, :, q:q + 36 * 7 + 1:36],
                            alpha[32 * q:32 * (q + 1), :ctc].rearrange(
                                "p (g t) -> p g t", t=8))

                    # ---- weighted sum: out_ps[32jj+m, d] += A.T @ nb
                    for g in range(ng_c):
                        jj = gglob % 4
                        if jj == 0:
                            out_ps = psop.tile([128, D], FP32, tag="out_ps")
                        for t in range(8):
                            nc.tensor.matmul(
                                out_ps[32 * jj:32 * (jj + 1), :],
                                A[:, 256 * g + 32 * t: 256 * g + 32 * (t + 1)],
                                nb[:, (8 * g + t) * D:(8 * g + t + 1) * D],
                                start=(t == 0), stop=(t == 7),
                                tile_position=(0, 32 * jj))
                        gglob += 1
                        # fill complete at jj==3 or last group overall
                        if jj == 3 or gglob == ngrp:
                            nrows = 32 * (jj + 1)
                            if osb is None:
                                osb = osbp.tile([128, OSB_FILLS * D], FP32,
                                                tag="osb")
                                osb_base = (gglob - 1) // 4 * 128
                            fi = ((gglob - 1) // 4 * 128 - osb_base) // 128
                            nc.scalar.copy(
                                osb[:nrows, fi * D:(fi + 1) * D],
                                out_ps[:nrows, :])
                            fill_nodes = osb_base + fi * 128 + nrows
                            if fi == OSB_FILLS - 1 or gglob == ngrp:
                                nv_rows = fill_nodes - osb_base
                                nfull = nv_rows // 128
                                if nfull:
                                    nc.sync.dma_start(
                                        out[osb_base: osb_base + nfull * 128]
                                        .rearrange("(f p) d -> p f d", p=128),
                                        osb[:, :nfull * D].rearrange(
                                            "p (f d) -> p f d", d=D))
                                rem = nv_rows % 128
                                if rem:
                                    nc.sync.dma_start(
                                        out[osb_base + nfull * 128: fill_nodes],
                                        osb[:rem, nfull * D:(nfull + 1) * D])
                                osb = None
    nc.compile()
    return nc


# ---------------------------------------------------------------- host side
def _fit_poly(W3, W4, b4):
    """coef columns: [b_DEG, ..., b_1, a_0 + b4] for r=(r+b_j)*x Horner."""
    w3 = np.asarray(W3, np.float64).reshape(-1)
    w4 = np.asarray(W4, np.float64).reshape(-1)
    xs = (1 - np.cos(np.linspace(0, np.pi, 4 * (DEG + 1)))) / 2  # cheb pts [0,1]
    ys = (1.0 / (1.0 + np.exp(-np.outer(xs, w3)))) @ w4
    cf = np.polynomial.chebyshev.Chebyshev.fit(xs, ys, DEG, domain=[0, 1])
    pw = cf.convert(kind=np.polynomial.polynomial.Polynomial)
    a = np.zeros(DEG + 1)
    a[: len(pw.coef)] = pw.coef          # a[j] = coeff of x^j
    xt = np.linspace(0, 1, 3001)
    yt = (1.0 / (1.0 + np.exp(-np.outer(xt, w3)))) @ w4
    resid = np.abs(np.polynomial.polynomial.polyval(xt, a) - yt).max()
    cols = [a[j] for j in range(DEG, 0, -1)] + [a[0] + float(np.asarray(b4).reshape(-1)[0])]
    coef = np.tile(np.asarray(cols, np.float32)[None, :], (128, 1))
    return coef, resid


def make_consts(W1, W2, W3, W4, b4):
    v1 = (np.asarray(W1) @ np.asarray(W4)).reshape(-1).astype(np.float32)
    v2 = (np.asarray(W2) @ np.asarray(W4)).reshape(-1).astype(np.float32)
    coef, resid = _fit_poly(W3, W4, b4)
    r = np.arange(128)
    e4 = (r[:, None] // 32 == np.arange(4)[None, :]).astype(np.float32)
    return {
        "v1rep": np.tile(v1[None, :], (128, 1)),
        "v2rep": np.tile(v2[None, :], (128, 1)),
        "coef": coef,
        "e4": e4,
        "e4b": e4.T.copy(),
        "eye": np.eye(128, dtype=np.float32),
    }, resid


_prog_cache = {}


def _get_prog(npc, ct):
    key = (npc, ct)
    if key not in _prog_cache:
        _install_neff_cache()
        _prog_cache[key] = build_program(npc, ct)
    return _prog_cache[key]


def kernel(self_vecs, neigh_vecs, neigh_deltatime, W1, W2, W3, W4, b4):
    self_vecs = np.asarray(self_vecs, np.float32)
    neigh_vecs = np.asarray(neigh_vecs, np.float32)
    neigh_deltatime = np.asarray(neigh_deltatime, np.float32)
    n = self_vecs.shape[0]
    ntot = NCORES * NPC

    sv = np.zeros((ntot, D), np.float32)
    sv[:n] = self_vecs
    nv = np.zeros((ntot * K, D), np.float32)
    nv[: n * K] = neigh_vecs.reshape(n * K, D)
    dtp = np.zeros((ntot * K,), np.float32)
    dtp[: n * K] = neigh_deltatime.reshape(-1)

    consts, resid = make_consts(W1, W2, W3, W4, b4)
    if resid > 2e-4:
        raise RuntimeError(f"poly fit residual too large: {resid}")

    in_maps = []
    for c in range(NCORES):
        m = dict(consts)
        m["selfv"] = sv[c * NPC:(c + 1) * NPC]
        m["neigh"] = nv[c * NPC * K:(c + 1) * NPC * K]
        m["dt"] = dtp[c * NPC * K:(c + 1) * NPC * K]
        in_maps.append(m)

    prog = _get_prog(NPC, 56)
    res = run_bass_kernel_spmd(prog, in_maps, core_ids=list(range(NCORES)))
    out = np.concatenate([res.results[c]["out"] for c in range(NCORES)], axis=0)
    sc = np.concatenate(
        [res.results[c]["score"].reshape(NPC, K) for c in range(NCORES)], axis=0)
    return out[:n], sc[:n]


# revision 9
# speedup vs baseline: 1.4343x; 1.4343x over previous
"""Trainium2 Bass kernel for nn_AttentionAggregator.

Math (per node n, K=32 neighbors, D=128):
  q = self @ W1; m = neigh @ W2; t = sigmoid(dt*W3)
  score = relu((q+m+t) @ W4 + b4); alpha = softmax_k(score)
  out = sum_k alpha * neigh;  returns (out, alpha)

Key factoring: (q+m+t)@W4 = self@(W1@W4) + neigh@(W2@W4) + f(dt)
  where f(x) = sum_e sigmoid(x*W3[e])*W4[e]  (scalar function, poly-fit on host).
So the only full-size work on the big [N,K,D] tensor is:
  s2 = neigh . v2  (DVE mult + segmented reduce), and
  out = alpha-weighted sum over k (PE block-diag matmuls with PSUM accum).
Data-parallel over nodes across 8 cores; everything per-core is independent.
"""

import hashlib
import os
import shutil
import numpy as np

import concourse.bass as bass
import concourse.tile as tile
from concourse import bacc, mybir
from concourse.bass_utils import run_bass_kernel_spmd

FP32 = mybir.dt.float32
AX = mybir.AxisListType
OP = mybir.AluOpType
AF = mybir.ActivationFunctionType

N_FULL, K, D = 50000, 32, 128
NCORES = 8
NPC = 6272            # nodes per core (pad 50000 -> 50176)
DEG = 14              # poly degree for f(dt)

# ---------------------------------------------------------------- NEFF cache
_CACHE_DIR = os.path.join(os.path.expanduser("~"), ".bass_neff_cache")


def _install_neff_cache():
    import concourse.bass2jax as b2j
    if getattr(b2j, "_neff_cache_installed", False):
        return
    os.makedirs(_CACHE_DIR, exist_ok=True)
    real = b2j.compile_bir_kernel

    def cached(ant_bir_str, compile_dir_path, neff_name="kernel.neff", **kw):
        key = hashlib.sha256(
            ant_bir_str if isinstance(ant_bir_str, bytes) else ant_bir_str.encode()
        ).hexdigest()[:32]
        cpath = os.path.join(_CACHE_DIR, f"{key}.neff")
        dst = os.path.join(compile_dir_path, neff_name)
        if os.path.exists(cpath):
            shutil.copy(cpath, dst)
            return dst
        f = real(ant_bir_str, compile_dir_path, neff_name=neff_name, **kw)
        try:
            shutil.copy(f, cpath)
        except OSError:
            pass
        return f

    b2j.compile_bir_kernel = cached
    b2j._neff_cache_installed = True


# ---------------------------------------------------------------- program
def build_program(npc=NPC, ct=56):
    """One SPMD program, identical on all cores. npc nodes/core, ct nk-tiles
    per chunk. Requires npc % 32 == 0, ct % 8 == 0."""
    assert npc % 32 == 0 and ct % 8 == 0
    nkpc = npc * K
    nt = nkpc // 128          # nk-tiles of 128 rows
    assert nt % 8 == 0
    ngrp = nt // 8            # groups of 8 tiles = 32 nodes
    u_full, u_rem = npc // 128, npc % 128   # node-tiles for self
    dtf, dt_remp = nkpc // 16384, (nkpc % 16384) // 128

    nc = bacc.Bacc("TRN2", target_bir_lowering=False, debug=False,
                   num_devices=NCORES)

    neigh = nc.dram_tensor("neigh", [nkpc, D], FP32, kind="ExternalInput").ap()
    selfv = nc.dram_tensor("selfv", [npc, D], FP32, kind="ExternalInput").ap()
    dt_in = nc.dram_tensor("dt", [nkpc], FP32, kind="ExternalInput").ap()
    v1rep = nc.dram_tensor("v1rep", [128, D], FP32, kind="ExternalInput").ap()
    v2rep = nc.dram_tensor("v2rep", [128, D], FP32, kind="ExternalInput").ap()
    coef = nc.dram_tensor("coef", [128, DEG + 1], FP32, kind="ExternalInput").ap()
    e4 = nc.dram_tensor("e4", [128, 4], FP32, kind="ExternalInput").ap()
    e4b = nc.dram_tensor("e4b", [4, 128], FP32, kind="ExternalInput").ap()
    eye = nc.dram_tensor("eye", [128, 128], FP32, kind="ExternalInput").ap()

    cscratch = nc.dram_tensor("cscratch", [npc], FP32).ap()
    out = nc.dram_tensor("out", [npc, D], FP32, kind="ExternalOutput").ap()
    score = nc.dram_tensor("score", [nkpc], FP32, kind="ExternalOutput").ap()

    with tile.TileContext(nc) as tc:
        with tc.tile_pool(name="persist", bufs=1) as pp:
            # ---- constants to SBUF
            v1s = pp.tile([128, D], FP32)
            nc.sync.dma_start(v1s[:], v1rep[:])
            v2s = pp.tile([128, D], FP32)
            nc.sync.dma_start(v2s[:], v2rep[:])
            cfs = pp.tile([128, DEG + 1], FP32)
            nc.sync.dma_start(cfs[:], coef[:])
            e4s = pp.tile([128, 4], FP32)
            nc.sync.dma_start(e4s[:], e4[:])
            e4bs = pp.tile([4, 128], FP32)
            nc.sync.dma_start(e4bs[:], e4b[:])
            eyes = pp.tile([128, 128], FP32)
            nc.sync.dma_start(eyes[:], eye[:])

            gsb = pp.tile([128, nt], FP32)       # g = c_rep + f(dt) + a0, score layout
            a_bufs = [pp.tile([128, (ct // 8) * 256], FP32, tag=f"A{i}",
                              name=f"Abuf{i}") for i in range(2)]

            # ================= phase 0: g = poly(dt^T) + broadcast(self.v1) =====
            with tc.tile_pool(name="ph0", bufs=1) as p0, \
                 tc.tile_pool(name="ph0ps", bufs=1, space="PSUM") as p0ps:
                # ---- c = self . v1  (per node)
                u_cols = u_full + (1 if u_rem else 0)
                csb = p0.tile([128, u_cols], FP32, tag="cnat")
                nc.vector.memset(csb[:], 0.0)
                if u_full:
                    ssb = p0.tile([128, u_full * D], FP32, tag="selfsb")
                    nc.sync.dma_start(
                        ssb[:].rearrange("p (u d) -> p u d", d=D),
                        selfv[: u_full * 128].rearrange("(u p) d -> p u d", p=128))
                    smul = p0.tile([128, u_full * D], FP32, tag="smul")
                    nc.vector.tensor_tensor(
                        smul[:].rearrange("p (u d) -> p u d", d=D),
                        ssb[:].rearrange("p (u d) -> p u d", d=D),
                        v1s[:].unsqueeze(1).broadcast_to([128, u_full, D]),
                        op=OP.mult)
                    nc.vector.tensor_reduce(
                        csb[:, :u_full],
                        smul[:].rearrange("p (u d) -> p u d", d=D),
                        axis=AX.X, op=OP.add)
                if u_rem:
                    ssb2 = p0.tile([u_rem, D], FP32, tag="selfsb2")
                    nc.sync.dma_start(ssb2[:], selfv[u_full * 128:])
                    smul2 = p0.tile([u_rem, D], FP32, tag="smul2")
                    nc.vector.tensor_tensor(smul2[:], ssb2[:], v1s[:u_rem, :],
                                            op=OP.mult)
                    nc.vector.tensor_reduce(
                        csb[:u_rem, u_full:u_full + 1], smul2[:],
                        axis=AX.X, op=OP.add)
                # c4[j, t] = c[4t + j] via DRAM scratch (simple DMAs)
                if u_full:
                    nc.sync.dma_start(
                        cscratch[: u_full * 128].rearrange("(u p) -> p u", p=128),
                        csb[:, :u_full])
                if u_rem:
                    nc.sync.dma_start(cscratch[u_full * 128:],
                                      csb[:u_rem, u_full:u_full + 1])
                c4 = p0.tile([4, (nt + 31) // 32 * 32], FP32, tag="c4")
                nc.sync.dma_start(
                    c4[:, :nt],
                    cscratch[:].rearrange("(t j) -> j t", j=4))

                # ---- dt transpose to score layout, 128x128 blocks via PE
                pdt = p0ps.tile([128, 128], FP32, tag="pdt")
                dtx = p0.tile([128, nt], FP32, tag="dtx")
                if dtf:
                    dtn = p0.tile([128, dtf * 128], FP32, tag="dtn")
                    nc.sync.dma_start(
                        dtn[:].rearrange("p (t c) -> p t c", c=128),
                        dt_in[: dtf * 16384].rearrange("(t p c) -> p t c",
                                                       p=128, c=128))
                    for t in range(dtf):
                        pdt_t = p0ps.tile([128, 128], FP32, tag="pdt")
                        nc.tensor.matmul(pdt_t[:], dtn[:, t * 128:(t + 1) * 128],
                                         eyes[:], start=True, stop=True)
                        nc.scalar.copy(dtx[:, t * 128:(t + 1) * 128], pdt_t[:])
                if dt_remp:
                    dtn2 = p0.tile([dt_remp, 128], FP32, tag="dtn2")
                    nc.sync.dma_start(
                        dtn2[:],
                        dt_in[dtf * 16384:].rearrange("(p c) -> p c", c=128))
                    pdt2 = p0ps.tile([128, dt_remp], FP32, tag="pdt2")
                    nc.tensor.matmul(pdt2[:], dtn2[:],
                                     eyes[:dt_remp, :dt_remp],
                                     start=True, stop=True)
                    nc.scalar.copy(dtx[:, dtf * 128: dtf * 128 + dt_remp],
                                   pdt2[:])

                # ---- c broadcast to [128, nt] via PE: lhsT=e4b, rhs=c4
                pg = p0ps.tile([128, ((nt + 511) // 512) * 512], FP32, tag="pg")
                for s0 in range(0, nt, 512):
                    s1 = min(s0 + 512, nt)
                    nc.tensor.matmul(pg[:, s0:s1], e4bs[:], c4[:, s0:s1],
                                     start=True, stop=True)

                # ---- poly eval: r=0; r=(r+b_j)*x ...; g = (r + a0) + c_rep
                r0 = p0.tile([128, nt], FP32, tag="poly0")
                r1 = p0.tile([128, nt], FP32, tag="poly1")
                nc.vector.tensor_scalar_mul(r0[:], dtx[:], cfs[:, 0:1])
                cur, nxt = r0, r1
                for j in range(1, DEG):
                    nc.vector.scalar_tensor_tensor(
                        nxt[:], cur[:], cfs[:, j:j + 1], dtx[:],
                        op0=OP.add, op1=OP.mult)
                    cur, nxt = nxt, cur
                nc.vector.scalar_tensor_tensor(
                    gsb[:], cur[:], cfs[:, DEG:DEG + 1], pg[:, :nt],
                    op0=OP.add, op1=OP.add)

            # ================= phase 1: main streaming loop ====================
            with tc.tile_pool(name="nb", bufs=3) as nbp, \
                 tc.tile_pool(name="nv", bufs=2) as nvp, \
                 tc.tile_pool(name="sm", bufs=3) as smp, \
                 tc.tile_pool(name="osb", bufs=2) as osbp, \
                 tc.tile_pool(name="ps", bufs=2, space="PSUM") as psp, \
                 tc.tile_pool(name="pso", bufs=2, space="PSUM") as psop:

                for ab in a_bufs:
                    nc.vector.memset(ab[:], 0.0)

                nchunks = (nt + ct - 1) // ct
                gglob = 0           # global group index
                fill_nodes = 0      # nodes accumulated in current out batch
                osb = None
                osb_base = 0        # first node row of current osb batch
                OSB_FILLS = 8
                out_ps = None

                for ck in range(nchunks):
                    t0 = ck * ct
                    ctc = min(ct, nt - t0)          # tiles this chunk
                    ng_c = ctc // 8                 # groups this chunk
                    nb = nbp.tile([128, ct * D], FP32, tag="nb")
                    nc.sync.dma_start(
                        nb[:, :ctc * D].rearrange("p (t d) -> p t d", d=D),
                        neigh[t0 * 128:(t0 + ctc) * 128].rearrange(
                            "(t p) d -> p t d", p=128))

                    # ---- s2 = neigh . v2 (two halves)
                    s2 = smp.tile([128, ct], FP32, tag="s2")
                    half = ctc // 2
                    for h, (ha, hb) in enumerate(((0, half), (half, ctc))):
                        nh = hb - ha
                        nv = nvp.tile([128, (ct // 2 + 4) * D], FP32, tag="nv")
                        nc.vector.tensor_tensor(
                            nv[:, :nh * D].rearrange("p (t d) -> p t d", d=D),
                            nb[:, ha * D:hb * D].rearrange("p (t d) -> p t d", d=D),
                            v2s[:].unsqueeze(1).broadcast_to([128, nh, D]),
                            op=OP.mult)
                        nc.vector.tensor_reduce(
                            s2[:, ha:hb],
                            nv[:, :nh * D].rearrange("p (t d) -> p t d", d=D),
                            axis=AX.X, op=OP.add)

                    # ---- scores: spre = relu(s2 + g); e = exp(spre)
                    spre = smp.tile([128, ct], FP32, tag="spre")
                    nc.vector.tensor_tensor(spre[:, :ctc], s2[:, :ctc],
                                            gsb[:, t0:t0 + ctc], op=OP.add)
                    sprer = smp.tile([128, ct], FP32, tag="sprer")
                    nc.vector.tensor_scalar_max(sprer[:, :ctc], spre[:, :ctc], 0.0)
                    esb = smp.tile([128, ct], FP32, tag="esb")
                    nc.scalar.activation(esb[:, :ctc], sprer[:, :ctc], AF.Exp)

                    # ---- softmax denom: sums over k via PE mask matmul
                    ps_s = psp.tile([4, ct], FP32, tag="ps_s")
                    nc.tensor.matmul(ps_s[:, :ctc], e4s[:], esb[:, :ctc],
                                     start=True, stop=True)
                    inv = smp.tile([4, ct], FP32, tag="inv")
                    nc.vector.reciprocal(inv[:, :ctc], ps_s[:, :ctc])
                    ps_b = psp.tile([128, ct], FP32, tag="ps_b")
                    nc.tensor.matmul(ps_b[:, :ctc], e4bs[:], inv[:, :ctc],
                                     start=True, stop=True)
                    alpha = smp.tile([128, ct], FP32, tag="alpha")
                    nc.vector.tensor_tensor(alpha[:, :ctc], esb[:, :ctc],
                                            ps_b[:, :ctc], op=OP.mult)

                    # ---- score out: transpose alpha on PE so DRAM rows are
                    # 512B-contiguous (4B-scatter descriptors killed perf)
                    ps_at = psp.tile([ct, 128], FP32, tag="ps_at")
                    nc.tensor.matmul(ps_at[:ctc, :], alpha[:, :ctc], eyes[:],
                                     start=True, stop=True)
                    alphat = smp.tile([ct, 128], FP32, tag="alphat")
                    nc.scalar.copy(alphat[:ctc, :], ps_at[:ctc, :])
                    nc.sync.dma_start(
                        score[t0 * 128:(t0 + ctc) * 128].rearrange(
                            "(t p) -> t p", p=128),
                        alphat[:ctc, :])

                    # ---- A build: A[r, 256g + 36t + q] = alpha[r, 8g+t], q=r//32
                    A = a_bufs[ck % 2]
                    for q in range(4):
                        nc.vector.tensor_copy(
                            A[32 * q:32 * (q + 1), :ng_c * 256].rearrange(
                                "p (g j) -> p g j", j=256)[:, :, q:q + 36 * 7 + 1:36],
                            alpha[32 * q:32 * (q + 1), :ctc].rearrange(
                                "p (g t) -> p g t", t=8))

                    # ---- weighted sum: out_ps[32jj+m, d] += A.T @ nb
                    for g in range(ng_c):
                        jj = gglob % 4
                        if jj == 0:
                            out_ps = psop.tile([128, D], FP32, tag="out_ps")
                        for t in range(8):
                            nc.tensor.matmul(
                                out_ps[32 * jj:32 * (jj + 1), :],
                                A[:, 256 * g + 32 * t: 256 * g + 32 * (t + 1)],
                                nb[:, (8 * g + t) * D:(8 * g + t + 1) * D],
                                start=(t == 0), stop=(t == 7),
                                tile_position=(0, 32 * jj))
                        gglob += 1
                        # fill complete at jj==3 or last group overall
                        if jj == 3 or gglob == ngrp:
                            nrows = 32 * (jj + 1)
                            if osb is None:
                                osb = osbp.tile([128, OSB_FILLS * D], FP32,
                                                tag="osb")
                                osb_base = (gglob - 1) // 4 * 128
                            fi = ((gglob - 1) // 4 * 128 - osb_base) // 128
                            nc.scalar.copy(
                                osb[:nrows, fi * D:(fi + 1) * D],
                                out_ps[:nrows, :])
                            fill_nodes = osb_base + fi * 128 + nrows
                            if fi == OSB_FILLS - 1 or gglob == ngrp:
                                nv_rows = fill_nodes - osb_base
                                nfull = nv_rows // 128
                                if nfull:
                                    nc.sync.dma_start(
                                        out[osb_base: osb_base + nfull * 128]
                                        .rearrange("(f p) d -> p f d", p=128),
                                        osb[:, :nfull * D].rearrange(
                                            "p (f d) -> p f d", d=D))
                                rem = nv_rows % 128
                                if rem:
                                    nc.sync.dma_start(
                                        out[osb_base + nfull * 128: fill_nodes],
                                        osb[:rem, nfull * D:(nfull + 1) * D])
                                osb = None
    nc.compile()
    return nc


# ---------------------------------------------------------------- host side
def _fit_poly(W3, W4, b4):
    """coef columns: [b_DEG, ..., b_1, a_0 + b4] for r=(r+b_j)*x Horner."""
    w3 = np.asarray(W3, np.float64).reshape(-1)
    w4 = np.asarray(W4, np.float64).reshape(-1)
    xs = (1 - np.cos(np.linspace(0, np.pi, 4 * (DEG + 1)))) / 2  # cheb pts [0,1]
    ys = (1.0 / (1.0 + np.exp(-np.outer(xs, w3)))) @ w4
    cf = np.polynomial.chebyshev.Chebyshev.fit(xs, ys, DEG, domain=[0, 1])
    pw = cf.convert(kind=np.polynomial.polynomial.Polynomial)
    a = np.zeros(DEG + 1)
    a[: len(pw.coef)] = pw.coef          # a[j] = coeff of x^j
    xt = np.linspace(0, 1, 3001)
    yt = (1.0 / (1.0 + np.exp(-np.outer(xt, w3)))) @ w4
    resid = np.abs(np.polynomial.polynomial.polyval(xt, a) - yt).max()
    cols = [a[j] for j in range(DEG, 0, -1)] + [a[0] + float(np.asarray(b4).reshape(-1)[0])]
    coef = np.tile(np.asarray(cols, np.float32)[None, :], (128, 1))
    return coef, resid


def make_consts(W1, W2, W3, W4, b4):
    v1 = (np.asarray(W1) @ np.asarray(W4)).reshape(-1).astype(np.float32)
    v2 = (np.asarray(W2) @ np.asarray(W4)).reshape(-1).astype(np.float32)
    coef, resid = _fit_poly(W3, W4, b4)
    r = np.arange(128)
    e4 = (r[:, None] // 32 == np.arange(4)[None, :]).astype(np.float32)
    return {
        "v1rep": np.tile(v1[None, :], (128, 1)),
        "v2rep": np.tile(v2[None, :], (128, 1)),
        "coef": coef,
        "e4": e4,
        "e4b": e4.T.copy(),
        "eye": np.eye(128, dtype=np.float32),
    }, resid


_prog_cache = {}


def _get_prog(npc, ct):
    key = (npc, ct)
    if key not in _prog_cache:
        _install_neff_cache()
        _prog_cache[key] = build_program(npc, ct)
    return _prog_cache[key]


def kernel(self_vecs, neigh_vecs, neigh_deltatime, W1, W2, W3, W4, b4):
    self_vecs = np.asarray(self_vecs, np.float32)
    neigh_vecs = np.asarray(neigh_vecs, np.float32)
    neigh_deltatime = np.asarray(neigh_deltatime, np.float32)
    n = self_vecs.shape[0]
    ntot = NCORES * NPC

    sv = np.zeros((ntot, D), np.float32)
    sv[:n] = self_vecs
    nv = np.zeros((ntot * K, D), np.float32)
    nv[: n * K] = neigh_vecs.reshape(n * K, D)
    dtp = np.zeros((ntot * K,), np.float32)
    dtp[: n * K] = neigh_deltatime.reshape(-1)

    consts, resid = make_consts(W1, W2, W3, W4, b4)
    if resid > 2e-4:
        raise RuntimeError(f"poly fit residual too large: {resid}")

    in_maps = []
    for c in range(NCORES):
        m = dict(consts)
        m["selfv"] = sv[c * NPC:(c + 1) * NPC]
        m["neigh"] = nv[c * NPC * K:(c + 1) * NPC * K]
        m["dt"] = dtp[c * NPC * K:(c + 1) * NPC * K]
        in_maps.append(m)

    prog = _get_prog(NPC, 56)
    res = run_bass_kernel_spmd(prog, in_maps, core_ids=list(range(NCORES)))
    out = np.concatenate([res.results[c]["out"] for c in range(NCORES)], axis=0)
    sc = np.concatenate(
        [res.results[c]["score"].reshape(NPC, K) for c in range(NCORES)], axis=0)
    return out[:n], sc[:n]
